# revision 11
# baseline (speedup 1.0000x reference)
"""CLUB loss kernel for Trainium2 (8 NeuronCores, SPMD row-sharded).

Math: the reference returns mean_i(pos_i - neg_i), a scalar.  Both the
pos and neg terms collapse into sums that never materialize the NxN
distance matrix:

  mean_pos = -0.5/N * (A - 2B + C)
      A = sum_{i,d} x[i,d]^2 * invv[i,d]
      B = sum_{i,d} x[i,d] * mu[i,d] * invv[i,d]
      C = sum_{i,d} mu[i,d]^2 * invv[i,d]
  mean_neg = -0.5 * (S_invv . S_x2 - 2 * S_muinvv . S_x + N*C) / N^2
      S_invv = sum_i invv[i,:]     S_muinvv = sum_i mu[i,:]*invv[i,:]
      S_x    = sum_j x[j,:]        S_x2     = sum_j x[j,:]^2
  loss = mean_pos - mean_neg

Each core handles 2048 rows (2 batches of x + matching mu/logvar rows)
and emits f32 partial sums; the host combines them in float64.

Layout: d-major (128, 1024): partition q = (sub-slab b, dim d), free
axis = row index.  Every reduction is a free-axis row-sum that rides as
an accum_out on the op that produces (or merely streams) the tensor.

v2 structure (from trace analysis of the 23us baseline):
- 6 half-tensor DMAs spread over THREE HWDGE queues (SP, ACT, DVE) so
  the 1.5MB input streams at the ~330GB/s two-queue-plus rate instead
  of one queue's ~190GB/s; triggers are issued before any compute so
  the ~650ns DMA_SEQ time never blocks a compute engine mid-chain.
- 13 compute instructions balanced across ACT (exp, sq, Sx-copy),
  DVE (muinvv, B, A) and Pool (C) - the baseline's 9-instruction ACT
  chain (with 9 x 278ns accumulator reads) was the compute bottleneck.
- one tile pool, per-engine scratch tiles, no memsets: fewer
  cross-engine sync events shrinks the multi-microsecond semaphore
  teardown tail that counts toward exec time.
"""

import sys

sys.path.insert(0, "/opt/trn_rl_repo")

import numpy as np
from contextlib import ExitStack

import concourse.bass as bass
import concourse.bacc as bacc
import concourse.tile as tile
from concourse import mybir
from concourse.bass_utils import run_bass_kernel_spmd

F32 = mybir.dt.float32
N_CORES = 8
B, D, H, W = 16, 64, 32, 32
HW = H * W                # 1024
N = B * HW                # 16384
NB = B // N_CORES         # 2 sub-slabs (batches) per core
ROWS = NB * HW            # 2048 rows per core
COLS = HW                 # free size of the (128, 1024) layout
HALF = COLS // 2

# accumulator column map: name -> [h0 col, h1 col].  The two sums that
# finish last (C h1, A h1) sit in the final columns so the split output
# DMA ships cols [0,12) early and only cols [12,14) after the last op.
ACC_COLS = {
    "Sinvv": [0, 1],
    "Sx2": [2, 3],
    "Sx": [4, 5],
    "Smuinvv": [6, 7],
    "B": [8, 9],
    "A": [10, 13],
    "C": [11, 12],
}
NACC = 14
NACC1 = 12  # cols [0, NACC1) go in the early output DMA

def build_nc() -> bass.Bass:
    nc = bacc.Bacc()
    ins = {
        nm: nc.dram_tensor(nm, [128, HALF], F32, kind="ExternalInput")
        for nm in ("lv0", "lv1", "mu0", "mu1", "x0", "x1")
    }
    accs = nc.dram_tensor("accs", [128, NACC], F32, kind="ExternalOutput")

    with ExitStack() as ctx:
        tc = ctx.enter_context(tile.TileContext(nc))
        pool = ctx.enter_context(tc.tile_pool(name="p", bufs=1))

        lv = pool.tile([128, COLS], F32)
        mu = pool.tile([128, COLS], F32)
        xb = pool.tile([128, COLS], F32)
        invv = pool.tile([128, COLS], F32)
        muinvv = pool.tile([128, COLS], F32)
        x2 = pool.tile([128, COLS], F32)
        cmul = pool.tile([128, COLS], F32)  # mu * muinvv (Pool product)
        gA = pool.tile([128, COLS], F32)    # ACT scratch
        gD = pool.tile([128, COLS], F32)    # DVE scratch
        acc = pool.tile([128, NACC], F32)

        HS = [slice(0, HALF), slice(HALF, COLS)]

        def col(q, h):
            c = ACC_COLS[q][h]
            return acc[:, c:c + 1]

        # Strict-order pins: each emitted op gets a min-sim-time 50us
        # after the previous one, so the tile scheduler cannot reorder
        # anything (its own DMA-latency model mispredicts SWDGE vs HWDGE
        # arrival order and otherwise causes head-of-line blocking on
        # the in-order engine queues).  Sim-time only - no HW waits.
        pin_t = [0.0]

        def pin():
            pin_t[0] += 50.0
            tc.tile_set_cur_wait(pin_t[0] / 1000.0)

        EXP = mybir.ActivationFunctionType.Exp
        SQ = mybir.ActivationFunctionType.Square
        CP = mybir.ActivationFunctionType.Copy
        M = mybir.AluOpType.mult
        X = mybir.AxisListType.X
        ADD = mybir.AluOpType.add

        def act(fn, out, in_, q, h, scale=1.0):
            nc.scalar.activation(
                out=out, in_=in_, func=fn, bias=0.0, scale=scale,
                accum_out=col(q, h),
            )

        def stt(out, in0, in1, q, h):
            nc.vector.scalar_tensor_tensor(
                out=out, in0=in0, scalar=1.0, in1=in1, op0=M, op1=M,
                accum_out=col(q, h),
            )

        # DMA triggers first.  512KB per queue (balanced ends, measured
        # aggregate ~340GB/s): SP lv halves, ACT-q mu halves, SWDGE x.
        pin()
        nc.sync.dma_start(out=lv[:, HS[0]], in_=ins["lv0"][:, :])
        pin()
        nc.sync.dma_start(out=lv[:, HS[1]], in_=ins["lv1"][:, :])
        pin()
        nc.scalar.dma_start(out=mu[:, HS[0]], in_=ins["mu0"][:, :])
        pin()
        nc.scalar.dma_start(out=mu[:, HS[1]], in_=ins["mu1"][:, :])
        pin()
        nc.gpsimd.dma_start(out=xb[:, HS[0]], in_=ins["x0"][:, :])
        pin()
        nc.gpsimd.dma_start(out=xb[:, HS[1]], in_=ins["x1"][:, :])

        # Compute, emitted in global pin order (producers before
        # consumers so the dep tracker sees every edge).  Measured
        # arrivals: lv0 ~10.3us, mu0 ~11.5, x0 ~12.5, lv1 ~13.1,
        # mu1 ~13.5, x1 ~14.1.
        pin()
        act(EXP, invv[:, HS[0]], lv[:, HS[0]], "Sinvv", 0, scale=-1.0)
        pin()
        stt(muinvv[:, HS[0]], mu[:, HS[0]], invv[:, HS[0]], "Smuinvv", 0)
        pin()
        nc.gpsimd.tensor_mul(cmul[:, HS[0]], mu[:, HS[0]], muinvv[:, HS[0]])
        pin()
        stt(gD[:, HS[0]], xb[:, HS[0]], muinvv[:, HS[0]], "B", 0)
        pin()
        act(EXP, invv[:, HS[1]], lv[:, HS[1]], "Sinvv", 1, scale=-1.0)
        pin()
        act(SQ, x2[:, HS[0]], xb[:, HS[0]], "Sx2", 0)
        pin()
        stt(muinvv[:, HS[1]], mu[:, HS[1]], invv[:, HS[1]], "Smuinvv", 1)
        pin()
        nc.gpsimd.tensor_mul(cmul[:, HS[1]], mu[:, HS[1]], muinvv[:, HS[1]])
        pin()
        stt(gD[:, HS[1]], xb[:, HS[1]], muinvv[:, HS[1]], "B", 1)
        pin()
        act(SQ, x2[:, HS[1]], xb[:, HS[1]], "Sx2", 1)
        pin()
        stt(gD[:, HS[0]], x2[:, HS[0]], invv[:, HS[0]], "A", 0)
        pin()
        nc.vector.tensor_reduce(
            out=col("Sx", 1), in_=xb[:, HS[1]], axis=X, op=ADD)
        pin()
        act(CP, gA[:, HS[0]], xb[:, HS[0]], "Sx", 0)
        pin()
        act(CP, gA[:, HS[1]], cmul[:, HS[0]], "C", 0)
        pin()
        nc.vector.tensor_reduce(
            out=col("C", 1), in_=cmul[:, HS[1]], axis=X, op=ADD)
        pin()
        # early output DMA: cols [0,12) are complete before A1 lands
        nc.sync.dma_start(out=accs[:, 0:NACC1], in_=acc[:, 0:NACC1])
        pin()
        stt(gD[:, HS[1]], x2[:, HS[1]], invv[:, HS[1]], "A", 1)
        pin()
        nc.sync.dma_start(out=accs[:, NACC1:NACC], in_=acc[:, NACC1:NACC])
    return nc


def _ensure_ntff_hook():
    """This image's antenv lacks axon_hooks; if tracing is requested
    (e.g. BASS_TRACE=1), run_bass_kernel_spmd would die on the import.
    Register the ctypes-based hook if available, else a None hook so
    tracing degrades gracefully."""
    import types

    if "antenv.axon_hooks" in sys.modules:
        return
    try:
        import antenv.axon_hooks  # noqa: F401
        return
    except ImportError:
        pass
    hook = None
    try:
        sys.path.insert(0, "/root/.axon_site")
        from trn_agent_boot.trn_boot import _ntff_profile_via_ctypes

        hook = _ntff_profile_via_ctypes("/opt/axon/libaxon_pjrt.so")
    except Exception:
        hook = None
    mod = types.ModuleType("antenv.axon_hooks")
    mod._hook = hook
    mod.get_axon_ntff_profile_hook = lambda: mod._hook
    mod.set_axon_ntff_profile_hook = lambda h: setattr(mod, "_hook", h)
    sys.modules["antenv.axon_hooks"] = mod


_ensure_ntff_hook()

_NC = None


def _get_nc():
    global _NC
    if _NC is None:
        _NC = build_nc()
        # bacc passes legalize multi-sync-wait instructions for TRN2 codegen
        _NC.compile()
    return _NC


def make_in_maps(x, mu, logvar):
    x = np.ascontiguousarray(np.asarray(x, dtype=np.float32))
    mu = np.asarray(mu, dtype=np.float32)
    lv = np.asarray(logvar, dtype=np.float32)
    in_maps = []
    for c in range(N_CORES):
        r0 = c * ROWS
        mu_t = np.concatenate(
            [mu[r0 + b * HW:r0 + (b + 1) * HW].T for b in range(NB)], axis=0
        )
        lv_t = np.concatenate(
            [lv[r0 + b * HW:r0 + (b + 1) * HW].T for b in range(NB)], axis=0
        )
        x_t = x[c * NB:(c + 1) * NB].reshape(128, COLS)
        m = {}
        for h, sl in enumerate((slice(0, HALF), slice(HALF, COLS))):
            m[f"lv{h}"] = np.ascontiguousarray(lv_t[:, sl])
            m[f"mu{h}"] = np.ascontiguousarray(mu_t[:, sl])
            m[f"x{h}"] = np.ascontiguousarray(x_t[:, sl])
        in_maps.append(m)
    return in_maps


def combine(results) -> np.ndarray:
    tot = {q: np.zeros(128, dtype=np.float64) for q in ACC_COLS}
    for r in results:
        a = np.asarray(r["accs"], dtype=np.float64)  # (128, NACC)
        for q, cols in ACC_COLS.items():
            tot[q] += a[:, cols].sum(axis=1)
    A, Bs, C = (tot[q].sum() for q in ("A", "B", "C"))
    vec = {q: tot[q].reshape(NB, D).sum(axis=0)
           for q in ("Sx", "Sx2", "Sinvv", "Smuinvv")}
    mean_pos = -0.5 / N * (A - 2.0 * Bs + C)
    mean_D = (vec["Sinvv"] @ vec["Sx2"] - 2.0 * vec["Smuinvv"] @ vec["Sx"]
              + N * C) / float(N) ** 2
    loss = mean_pos + 0.5 * mean_D
    return np.array(loss, dtype=np.float32)


def kernel(x, mu, logvar, **_kwargs):
    nc = _get_nc()
    in_maps = make_in_maps(x, mu, logvar)
    res = run_bass_kernel_spmd(nc, in_maps, list(range(N_CORES)))
    return combine(res.results)


# revision 12
# speedup vs baseline: 1.0305x; 1.0305x over previous
"""CLUB loss kernel for Trainium2 (8 NeuronCores, SPMD row-sharded).

Math: the reference returns mean_i(pos_i - neg_i), a scalar.  Both the
pos and neg terms collapse into sums that never materialize the NxN
distance matrix:

  mean_pos = -0.5/N * (A - 2B + C)
      A = sum_{i,d} x[i,d]^2 * invv[i,d]
      B = sum_{i,d} x[i,d] * mu[i,d] * invv[i,d]
      C = sum_{i,d} mu[i,d]^2 * invv[i,d]
  mean_neg = -0.5 * (S_invv . S_x2 - 2 * S_muinvv . S_x + N*C) / N^2
      S_invv = sum_i invv[i,:]     S_muinvv = sum_i mu[i,:]*invv[i,:]
      S_x    = sum_j x[j,:]        S_x2     = sum_j x[j,:]^2
  loss = mean_pos - mean_neg
  (A, B, C are full scalars; the S_* are per-d 64-vectors.)

Each core handles 2048 rows (2 batches of x + matching mu/logvar rows)
and emits f32 partial sums; the host combines them in float64.

Layout: everything lives in the d-major layout (128, 1024): partition
q = (sub-slab b, dim d), free axis = row index within the sub-slab.
x arrives in this layout naturally (x[b] is (d, h*w) row-major); mu and
logvar are pre-transposed on the host as part of the shard layout.
With d on partitions every needed reduction is a free-axis row-sum, so
each quantity is one fused elementwise+accumulate instruction - no
on-chip transposes, no PSUM, no TensorEngine work at all (~20 compute
instructions per core).
"""

import sys

sys.path.insert(0, "/opt/trn_rl_repo")

import numpy as np
from contextlib import ExitStack

import concourse.bass as bass
import concourse.bacc as bacc
import concourse.tile as tile
from concourse import mybir
from concourse.bass_utils import run_bass_kernel_spmd

F32 = mybir.dt.float32
N_CORES = 8
B, D, H, W = 16, 64, 32, 32
HW = H * W                # 1024
N = B * HW                # 16384
NB = B // N_CORES         # 2 sub-slabs (batches) per core
ROWS = NB * HW            # 2048 rows per core
COLS = HW                 # free size of the (128, 1024) layout
# accum column map: quantity q, chunk c -> column q*NCH + c
QUANT = ["A", "B", "C", "Sx", "Sx2", "Sinvv", "Smuinvv"]
# Asymmetric chunks: chunk-0 sized so DVE stays busy until the bulk
# chunk lands (it idled 1.7us at 256), tiny last chunk so the dependency
# chain after the final DMA completes is short.
BOUNDS = [0, 512, 896, 1024]
NCH = len(BOUNDS) - 1


def build_nc() -> bass.Bass:
    nc = bacc.Bacc()
    # one fully-contiguous DRAM tensor per (input, chunk) so every load is
    # a pure 1D burst (the host packs chunks during shard prep)
    xn, mut, lvt = ({
        h: nc.dram_tensor(f"{nm}{h}", [128, BOUNDS[h + 1] - BOUNDS[h]], F32,
                          kind="ExternalInput")
        for h in range(NCH)
    } for nm in ("xn", "mut", "lvt"))
    accs = nc.dram_tensor("accs", [128, len(QUANT) * NCH], F32,
                          kind="ExternalOutput")

    with ExitStack() as ctx:
        tc = ctx.enter_context(tile.TileContext(nc))
        big = ctx.enter_context(tc.tile_pool(name="big", bufs=1))
        jp = ctx.enter_context(tc.tile_pool(name="jp", bufs=2))
        accp = ctx.enter_context(tc.tile_pool(name="accp", bufs=1))

        zerob = big.tile([128, 1], F32)
        nc.scalar.memzero(zerob[:])

        xb = big.tile([128, COLS], F32)
        mu = big.tile([128, COLS], F32)
        lv = big.tile([128, COLS], F32)
        sls = [slice(BOUNDS[h], BOUNDS[h + 1]) for h in range(NCH)]
        # Split DMA issue across both HWDGE engines (SP + ACT) so the
        # descriptor generations don't serialize; per chunk, lv first (it
        # heads the exp -> muinvv -> B/C chain), then mu, then x.
        qs = [nc.sync, nc.scalar]
        qi = 0
        for h in range(NCH):
            for t_dram, t_sbuf in ((lvt, lv), (mut, mu), (xn, xb)):
                qs[qi % 2].dma_start(
                    out=t_sbuf[:, sls[h]], in_=t_dram[h][:, :]
                )
                qi += 1

        invv = big.tile([128, COLS], F32)
        muinvv = big.tile([128, COLS], F32)
        x2 = big.tile([128, COLS], F32)
        acc = accp.tile([128, len(QUANT) * NCH], F32)

        def col(q, c):
            return acc[:, QUANT.index(q) * NCH + c:QUANT.index(q) * NCH + c + 1]

        M = mybir.AluOpType.mult

        def act(q, h, out, in_, func, scale=1.0):
            nc.scalar.activation(
                out=out, in_=in_, func=func, bias=zerob[:], scale=scale,
                accum_out=col(q, h),
            )

        def stt(q, h, in0, in1):
            jd = jp.tile([128, BOUNDS[h + 1] - BOUNDS[h]], F32, tag="jd",
                         name=f"jd_{q}{h}")
            nc.vector.scalar_tensor_tensor(
                out=jd[:], in0=in0[:, sls[h]], scalar=1.0, in1=in1[:, sls[h]],
                op0=M, op1=M, accum_out=col(q, h),
            )

        EXP = mybir.ActivationFunctionType.Exp
        SQ = mybir.ActivationFunctionType.Square

        # Emission order = engine program order.  Per chunk: the lv/mu
        # chain ops and the x-gated ops; the GPS-gated Smuinvv copies go
        # last so they never stall the ACT program.
        for h in range(NCH):
            act("Sinvv", h, invv[:, sls[h]], lv[:, sls[h]], EXP, scale=-1.0)
            nc.gpsimd.tensor_mul(
                muinvv[:, sls[h]], mu[:, sls[h]], invv[:, sls[h]]
            )
            act("Sx2", h, x2[:, sls[h]], xb[:, sls[h]], SQ)
            jd = jp.tile([128, BOUNDS[h + 1] - BOUNDS[h]], F32, tag="jd",
                         name=f"jd_sx{h}")
            nc.vector.tensor_scalar(
                out=jd[:], in0=xb[:, sls[h]], scalar1=1.0, scalar2=0.0,
                op0=M, op1=mybir.AluOpType.add, accum_out=col("Sx", h),
            )
            stt("A", h, x2, invv)
            stt("C", h, mu, muinvv)
            stt("B", h, xb, muinvv)
        for h in range(NCH):
            ja = jp.tile([128, BOUNDS[h + 1] - BOUNDS[h]], F32, tag="ja",
                         name=f"ja_{h}")
            nc.scalar.activation(
                out=ja[:], in_=muinvv[:, sls[h]],
                func=mybir.ActivationFunctionType.Copy,
                bias=0.0, scale=1.0, accum_out=col("Smuinvv", h),
            )

        nc.sync.dma_start(out=accs[:, :], in_=acc[:])
    return nc


def _ensure_ntff_hook():
    """This image's antenv lacks axon_hooks; if tracing is requested
    (e.g. BASS_TRACE=1), run_bass_kernel_spmd would die on the import.
    Register the ctypes-based hook if available, else a None hook so
    tracing degrades gracefully."""
    import types

    if "antenv.axon_hooks" in sys.modules:
        return
    try:
        import antenv.axon_hooks  # noqa: F401
        return
    except ImportError:
        pass
    hook = None
    try:
        sys.path.insert(0, "/root/.axon_site")
        from trn_agent_boot.trn_boot import _ntff_profile_via_ctypes

        hook = _ntff_profile_via_ctypes("/opt/axon/libaxon_pjrt.so")
    except Exception:
        hook = None
    mod = types.ModuleType("antenv.axon_hooks")
    mod._hook = hook
    mod.get_axon_ntff_profile_hook = lambda: mod._hook
    mod.set_axon_ntff_profile_hook = lambda h: setattr(mod, "_hook", h)
    sys.modules["antenv.axon_hooks"] = mod


_ensure_ntff_hook()

_NC = None


def _get_nc():
    global _NC
    if _NC is None:
        _NC = build_nc()
        # bacc passes legalize multi-sync-wait instructions for TRN2 codegen
        _NC.compile()
    return _NC


def make_in_maps(x, mu, logvar):
    x = np.ascontiguousarray(np.asarray(x, dtype=np.float32))
    mu = np.asarray(mu, dtype=np.float32)
    lv = np.asarray(logvar, dtype=np.float32)
    in_maps = []
    for c in range(N_CORES):
        r0 = c * ROWS
        mu_t = np.concatenate(
            [mu[r0 + b * HW:r0 + (b + 1) * HW].T for b in range(NB)], axis=0
        )
        lv_t = np.concatenate(
            [lv[r0 + b * HW:r0 + (b + 1) * HW].T for b in range(NB)], axis=0
        )
        x_t = x[c * NB:(c + 1) * NB].reshape(128, COLS)
        m = {}
        for h in range(len(BOUNDS) - 1):
            sl = slice(BOUNDS[h], BOUNDS[h + 1])
            m[f"xn{h}"] = np.ascontiguousarray(x_t[:, sl])
            m[f"mut{h}"] = np.ascontiguousarray(mu_t[:, sl])
            m[f"lvt{h}"] = np.ascontiguousarray(lv_t[:, sl])
        in_maps.append(m)
    return in_maps


def combine(results) -> np.ndarray:
    nq = len(QUANT)
    tot = np.zeros((nq, 128), dtype=np.float64)
    for r in results:
        a = np.asarray(r["accs"], dtype=np.float64)  # (128, nq*NCH)
        for q in range(nq):
            tot[q] += a[:, q * NCH:(q + 1) * NCH].sum(axis=1)
    scal = {q: tot[i].sum() for i, q in enumerate(QUANT[:3])}
    vec = {q: tot[i].reshape(NB, D).sum(axis=0)
           for i, q in enumerate(QUANT) if i >= 3}
    A, Bs, C = scal["A"], scal["B"], scal["C"]
    mean_pos = -0.5 / N * (A - 2.0 * Bs + C)
    mean_D = (vec["Sinvv"] @ vec["Sx2"] - 2.0 * vec["Smuinvv"] @ vec["Sx"]
              + N * C) / float(N) ** 2
    loss = mean_pos + 0.5 * mean_D
    return np.array(loss, dtype=np.float32)


def kernel(x, mu, logvar, **_kwargs):
    nc = _get_nc()
    in_maps = make_in_maps(x, mu, logvar)
    res = run_bass_kernel_spmd(nc, in_maps, list(range(N_CORES)))
    return combine(res.results)


# revision 13
# speedup vs baseline: 1.0801x; 1.0482x over previous
"""CLUB loss kernel, raw-bass variant (no TileContext / tile scheduler).

Same math, chunking, and op schedule as the baseline tile kernel, but
with hand-placed semaphores.  This removes the tile-framework prologue
(pool barriers, ordering-mode block, COMPARE_BRANCH guards) that sits
between the first_useful_time anchor and the first DMA trigger, and
makes every engine's program order explicit.
"""

import sys

sys.path.insert(0, "/opt/trn_rl_repo")

import numpy as np

import concourse.bass as bass
import concourse.bacc as bacc
from concourse import mybir
from concourse.bass_utils import run_bass_kernel_spmd

F32 = mybir.dt.float32
N_CORES = 8
B, D, H, W = 16, 64, 32, 32
HW = H * W
N = B * HW
NB = B // N_CORES
ROWS = NB * HW
COLS = HW
QUANT = ["A", "B", "C", "Sx", "Sx2", "Sinvv", "Smuinvv"]
BOUNDS = [0, 512, 896, 1024]
NCH = len(BOUNDS) - 1


def build_nc() -> bass.Bass:
    nc = bacc.Bacc()
    xn, mut, lvt = ({
        h: nc.dram_tensor(f"{nm}{h}", [128, BOUNDS[h + 1] - BOUNDS[h]], F32,
                          kind="ExternalInput")
        for h in range(NCH)
    } for nm in ("xn", "mut", "lvt"))
    accs = nc.dram_tensor("accs", [128, len(QUANT) * NCH], F32,
                          kind="ExternalOutput")

    with nc.cleanup_on_exit():
        lv = nc.alloc_sbuf_tensor("lv", [128, COLS], F32)
        mu = nc.alloc_sbuf_tensor("mu", [128, COLS], F32)
        xb = nc.alloc_sbuf_tensor("xb", [128, COLS], F32)
        invv = nc.alloc_sbuf_tensor("invv", [128, COLS], F32)
        muinvv = nc.alloc_sbuf_tensor("muinvv", [128, COLS], F32)
        x2 = nc.alloc_sbuf_tensor("x2", [128, COLS], F32)
        # dedicated scratch per discarded-output op (engine execution is
        # serial, but CoreSim's race detector wants disjoint buffers)
        jds = {(q, h): nc.alloc_sbuf_tensor(
            f"jd_{q}{h}", [128, BOUNDS[h + 1] - BOUNDS[h]], F32)
            for q in ("Sx", "A", "C", "B") for h in range(NCH)}
        jas = {h: nc.alloc_sbuf_tensor(
            f"ja_{h}", [128, BOUNDS[h + 1] - BOUNDS[h]], F32)
            for h in range(NCH)}
        acc = nc.alloc_sbuf_tensor("acc", [128, len(QUANT) * NCH], F32)

        sls = [slice(BOUNDS[h], BOUNDS[h + 1]) for h in range(NCH)]
        csz = [BOUNDS[h + 1] - BOUNDS[h] for h in range(NCH)]

        # per-transfer DMA sems (completion order across a queue is not
        # architecturally guaranteed, so no shared counting sem)
        dsem = {k: nc.alloc_semaphore(f"d_{k}") for k in
                ("lv0", "lv1", "lv2", "mu0", "mu1", "mu2", "x0", "x1", "x2")}
        sa = nc.alloc_semaphore("sa")      # ACT op counter
        sg = nc.alloc_semaphore("sg")      # GPS muinvv counter
        sv = nc.alloc_semaphore("sv")      # DVE op counter
        so = nc.alloc_semaphore("so")      # accs DMA done

        def col(q, c):
            i = QUANT.index(q) * NCH + c
            return acc.ap()[:, i:i + 1]

        # DMA triggers: same queue split as the tile baseline
        # (sync: lv0, x0, mu1, lv2, x2; scalar: mu0, lv1, x1, mu2).
        order = [("lv", 0, nc.sync), ("mu", 0, nc.scalar),
                 ("x", 0, nc.sync), ("lv", 1, nc.scalar),
                 ("mu", 1, nc.sync), ("x", 1, nc.scalar),
                 ("lv", 2, nc.sync), ("mu", 2, nc.scalar),
                 ("x", 2, nc.sync)]
        dram = {"lv": lvt, "mu": mut, "x": xn}
        sbuf = {"lv": lv, "mu": mu, "x": xb}
        for nm, h, eng in order:
            eng.dma_start(
                out=sbuf[nm].ap()[:, sls[h]], in_=dram[nm][h][:, :]
            ).then_inc(dsem[f"{nm}{h}"], 16)

        EXP = mybir.ActivationFunctionType.Exp
        SQ = mybir.ActivationFunctionType.Square
        CP = mybir.ActivationFunctionType.Copy
        M = mybir.AluOpType.mult

        # ACT program: exp_h, sq_h per chunk, then the Smuinvv copies.
        # sa counts 1..9 in this order.
        for h in range(NCH):
            nc.scalar.wait_ge(dsem[f"lv{h}"], 16)
            nc.scalar.activation(
                out=invv.ap()[:, sls[h]], in_=lv.ap()[:, sls[h]], func=EXP,
                bias=0.0, scale=-1.0, accum_out=col("Sinvv", h),
            ).then_inc(sa, 1)
            nc.scalar.wait_ge(dsem[f"x{h}"], 16)
            nc.scalar.activation(
                out=x2.ap()[:, sls[h]], in_=xb.ap()[:, sls[h]], func=SQ,
                bias=0.0, scale=1.0, accum_out=col("Sx2", h),
            ).then_inc(sa, 1)
        for h in range(NCH):
            nc.scalar.wait_ge(sg, h + 1)
            nc.scalar.activation(
                out=jas[h].ap()[:, :], in_=muinvv.ap()[:, sls[h]],
                func=CP, bias=0.0, scale=1.0, accum_out=col("Smuinvv", h),
            ).then_inc(sa, 1)

        # GPS program: muinvv_h = mu_h * invv_h.
        for h in range(NCH):
            nc.gpsimd.wait_ge(dsem[f"mu{h}"], 16)
            nc.gpsimd.wait_ge(sa, 2 * h + 1)  # exp_h done
            nc.gpsimd.tensor_mul(
                muinvv.ap()[:, sls[h]], mu.ap()[:, sls[h]],
                invv.ap()[:, sls[h]]
            ).then_inc(sg, 1)

        # DVE program: per chunk Sx, A, C, B (baseline order).
        for h in range(NCH):
            nc.vector.wait_ge(dsem[f"x{h}"], 16)
            nc.vector.tensor_scalar(
                out=jds[("Sx", h)].ap()[:, :], in0=xb.ap()[:, sls[h]],
                scalar1=1.0, scalar2=0.0, op0=M, op1=mybir.AluOpType.add,
                accum_out=col("Sx", h),
            ).then_inc(sv, 1)
            nc.vector.wait_ge(sa, 2 * h + 2)  # sq_h done => x2_h, invv_h
            nc.vector.scalar_tensor_tensor(
                out=jds[("A", h)].ap()[:, :], in0=x2.ap()[:, sls[h]], scalar=1.0,
                in1=invv.ap()[:, sls[h]], op0=M, op1=M,
                accum_out=col("A", h),
            ).then_inc(sv, 1)
            nc.vector.wait_ge(sg, h + 1)      # muinvv_h done
            nc.vector.scalar_tensor_tensor(
                out=jds[("C", h)].ap()[:, :], in0=mu.ap()[:, sls[h]], scalar=1.0,
                in1=muinvv.ap()[:, sls[h]], op0=M, op1=M,
                accum_out=col("C", h),
            ).then_inc(sv, 1)
            nc.vector.scalar_tensor_tensor(
                out=jds[("B", h)].ap()[:, :], in0=xb.ap()[:, sls[h]], scalar=1.0,
                in1=muinvv.ap()[:, sls[h]], op0=M, op1=M,
                accum_out=col("B", h),
            ).then_inc(sv, 1)

        # Output: wait for every accumulator column, ship, then confirm.
        nc.sync.wait_ge(sa, 3 * NCH)
        nc.sync.wait_ge(sv, 4 * NCH)
        nc.sync.dma_start(out=accs[:, :], in_=acc.ap()[:, :]).then_inc(so, 16)
        nc.sync.wait_ge(so, 16)
        nc.all_engine_barrier()
    return nc


def _ensure_ntff_hook():
    import types

    if "antenv.axon_hooks" in sys.modules:
        return
    try:
        import antenv.axon_hooks  # noqa: F401
        return
    except ImportError:
        pass
    hook = None
    try:
        sys.path.insert(0, "/root/.axon_site")
        from trn_agent_boot.trn_boot import _ntff_profile_via_ctypes

        hook = _ntff_profile_via_ctypes("/opt/axon/libaxon_pjrt.so")
    except Exception:
        hook = None
    mod = types.ModuleType("antenv.axon_hooks")
    mod._hook = hook
    mod.get_axon_ntff_profile_hook = lambda: mod._hook
    mod.set_axon_ntff_profile_hook = lambda h: setattr(mod, "_hook", h)
    sys.modules["antenv.axon_hooks"] = mod


_ensure_ntff_hook()

_NC = None


def _get_nc():
    global _NC
    if _NC is None:
        _NC = build_nc()
        _NC.compile()
    return _NC


def make_in_maps(x, mu, logvar):
    x = np.ascontiguousarray(np.asarray(x, dtype=np.float32))
    mu = np.asarray(mu, dtype=np.float32)
    lv = np.asarray(logvar, dtype=np.float32)
    in_maps = []
    for c in range(N_CORES):
        r0 = c * ROWS
        mu_t = np.concatenate(
            [mu[r0 + b * HW:r0 + (b + 1) * HW].T for b in range(NB)], axis=0
        )
        lv_t = np.concatenate(
            [lv[r0 + b * HW:r0 + (b + 1) * HW].T for b in range(NB)], axis=0
        )
        x_t = x[c * NB:(c + 1) * NB].reshape(128, COLS)
        m = {}
        for h in range(len(BOUNDS) - 1):
            sl = slice(BOUNDS[h], BOUNDS[h + 1])
            m[f"xn{h}"] = np.ascontiguousarray(x_t[:, sl])
            m[f"mut{h}"] = np.ascontiguousarray(mu_t[:, sl])
            m[f"lvt{h}"] = np.ascontiguousarray(lv_t[:, sl])
        in_maps.append(m)
    return in_maps


def combine(results) -> np.ndarray:
    nq = len(QUANT)
    tot = np.zeros((nq, 128), dtype=np.float64)
    for r in results:
        a = np.asarray(r["accs"], dtype=np.float64)
        for q in range(nq):
            tot[q] += a[:, q * NCH:(q + 1) * NCH].sum(axis=1)
    scal = {q: tot[i].sum() for i, q in enumerate(QUANT[:3])}
    vec = {q: tot[i].reshape(NB, D).sum(axis=0)
           for i, q in enumerate(QUANT) if i >= 3}
    A, Bs, C = scal["A"], scal["B"], scal["C"]
    mean_pos = -0.5 / N * (A - 2.0 * Bs + C)
    mean_D = (vec["Sinvv"] @ vec["Sx2"] - 2.0 * vec["Smuinvv"] @ vec["Sx"]
              + N * C) / float(N) ** 2
    loss = mean_pos + 0.5 * mean_D
    return np.array(loss, dtype=np.float32)


def kernel(x, mu, logvar, **_kwargs):
    nc = _get_nc()
    in_maps = make_in_maps(x, mu, logvar)
    res = run_bass_kernel_spmd(nc, in_maps, list(range(N_CORES)))
    return combine(res.results)


# revision 17
# speedup vs baseline: 1.0849x; 1.0044x over previous
"""CLUB loss kernel, raw-bass variant (no TileContext / tile scheduler).

Same math, chunking, and op schedule as the baseline tile kernel, but
with hand-placed semaphores.  This removes the tile-framework prologue
(pool barriers, ordering-mode block, COMPARE_BRANCH guards) that sits
between the first_useful_time anchor and the first DMA trigger, and
makes every engine's program order explicit.
"""

import sys

sys.path.insert(0, "/opt/trn_rl_repo")

import numpy as np

import concourse.bass as bass
import concourse.bacc as bacc
from concourse import mybir
from concourse.bass_utils import run_bass_kernel_spmd

F32 = mybir.dt.float32
N_CORES = 8
B, D, H, W = 16, 64, 32, 32
HW = H * W
N = B * HW
NB = B // N_CORES
ROWS = NB * HW
COLS = HW
QUANT = ["A", "B", "C", "Sx", "Sx2", "Sinvv", "Smuinvv"]
BOUNDS = [0, 512, 896, 1024]
NCH = len(BOUNDS) - 1


def build_nc() -> bass.Bass:
    nc = bacc.Bacc()
    xn, mut, lvt = ({
        h: nc.dram_tensor(f"{nm}{h}", [128, BOUNDS[h + 1] - BOUNDS[h]], F32,
                          kind="ExternalInput")
        for h in range(NCH)
    } for nm in ("xn", "mut", "lvt"))
    # one extra accum column (21): Sinvv of the exp0b sub-chunk — exp of
    # chunk 0 is split in two so the GPS muinvv product starts earlier.
    accs = nc.dram_tensor("accs", [128, len(QUANT) * NCH + 1], F32,
                          kind="ExternalOutput")

    with nc.cleanup_on_exit():
        lv = nc.alloc_sbuf_tensor("lv", [128, COLS], F32)
        mu = nc.alloc_sbuf_tensor("mu", [128, COLS], F32)
        xb = nc.alloc_sbuf_tensor("xb", [128, COLS], F32)
        invv = nc.alloc_sbuf_tensor("invv", [128, COLS], F32)
        muinvv = nc.alloc_sbuf_tensor("muinvv", [128, COLS], F32)
        x2 = nc.alloc_sbuf_tensor("x2", [128, COLS], F32)
        # dedicated scratch per discarded-output op (engine execution is
        # serial, but CoreSim's race detector wants disjoint buffers)
        jds = {(q, h): nc.alloc_sbuf_tensor(
            f"jd_{q}{h}", [128, BOUNDS[h + 1] - BOUNDS[h]], F32)
            for q in ("Sx", "A", "C", "B") for h in range(NCH)}
        jas = {h: nc.alloc_sbuf_tensor(
            f"ja_{h}", [128, BOUNDS[h + 1] - BOUNDS[h]], F32)
            for h in range(NCH)}
        acc = nc.alloc_sbuf_tensor("acc", [128, len(QUANT) * NCH + 1], F32)

        sls = [slice(BOUNDS[h], BOUNDS[h + 1]) for h in range(NCH)]
        csz = [BOUNDS[h + 1] - BOUNDS[h] for h in range(NCH)]

        # per-transfer DMA sems (completion order across a queue is not
        # architecturally guaranteed, so no shared counting sem)
        dsem = {k: nc.alloc_semaphore(f"d_{k}") for k in
                ("lv0", "lv1", "lv2", "mu0", "mu1", "mu2", "x0", "x1", "x2")}
        sa = nc.alloc_semaphore("sa")      # ACT op counter
        sg = nc.alloc_semaphore("sg")      # GPS muinvv counter
        sv = nc.alloc_semaphore("sv")      # DVE op counter
        so = nc.alloc_semaphore("so")      # accs DMA done

        def col(q, c):
            i = QUANT.index(q) * NCH + c
            return acc.ap()[:, i:i + 1]

        # DMA triggers: same queue split as the tile baseline
        # (sync: lv0, x0, mu1, lv2, x2; scalar: mu0, lv1, x1, mu2).
        order = [("lv", 0, nc.sync), ("mu", 0, nc.scalar),
                 ("x", 0, nc.sync), ("lv", 1, nc.scalar),
                 ("mu", 1, nc.sync), ("x", 1, nc.scalar),
                 ("lv", 2, nc.sync), ("mu", 2, nc.scalar),
                 ("x", 2, nc.sync)]
        dram = {"lv": lvt, "mu": mut, "x": xn}
        sbuf = {"lv": lv, "mu": mu, "x": xb}
        for nm, h, eng in order:
            eng.dma_start(
                out=sbuf[nm].ap()[:, sls[h]], in_=dram[nm][h][:, :]
            ).then_inc(dsem[f"{nm}{h}"], 16)

        EXP = mybir.ActivationFunctionType.Exp
        SQ = mybir.ActivationFunctionType.Square
        CP = mybir.ActivationFunctionType.Copy
        M = mybir.AluOpType.mult

        # ACT program: exp0 split in two sub-chunks (0:256, 256:512) so
        # GPS can start the first muinvv product ~0.5us earlier; then
        # exp_h/sq_h per remaining chunk and the Smuinvv copies.
        # sa counts: exp0a=1, exp0b=2, sq0=3, exp1=4, sq1=5, exp2=6,
        # sq2=7, cp0=8, cp1=9, cp2=10.
        SUB = [slice(0, 256), slice(256, 512)]
        nc.scalar.wait_ge(dsem["lv0"], 16)
        nc.scalar.activation(
            out=invv.ap()[:, SUB[0]], in_=lv.ap()[:, SUB[0]], func=EXP,
            bias=0.0, scale=-1.0, accum_out=col("Sinvv", 0),
        ).then_inc(sa, 1)
        nc.scalar.activation(
            out=invv.ap()[:, SUB[1]], in_=lv.ap()[:, SUB[1]], func=EXP,
            bias=0.0, scale=-1.0, accum_out=acc.ap()[:, 21:22],
        ).then_inc(sa, 1)
        nc.scalar.wait_ge(dsem["x0"], 16)
        nc.scalar.activation(
            out=x2.ap()[:, sls[0]], in_=xb.ap()[:, sls[0]], func=SQ,
            bias=0.0, scale=1.0, accum_out=col("Sx2", 0),
        ).then_inc(sa, 1)
        for h in (1, 2):
            nc.scalar.wait_ge(dsem[f"lv{h}"], 16)
            nc.scalar.activation(
                out=invv.ap()[:, sls[h]], in_=lv.ap()[:, sls[h]], func=EXP,
                bias=0.0, scale=-1.0, accum_out=col("Sinvv", h),
            ).then_inc(sa, 1)
            nc.scalar.wait_ge(dsem[f"x{h}"], 16)
            nc.scalar.activation(
                out=x2.ap()[:, sls[h]], in_=xb.ap()[:, sls[h]], func=SQ,
                bias=0.0, scale=1.0, accum_out=col("Sx2", h),
            ).then_inc(sa, 1)
        for h in range(NCH):
            nc.scalar.wait_ge(sg, h + 2)  # full muinvv_h done
            nc.scalar.activation(
                out=jas[h].ap()[:, :], in_=muinvv.ap()[:, sls[h]],
                func=CP, bias=0.0, scale=1.0, accum_out=col("Smuinvv", h),
            ).then_inc(sa, 1)

        # GPS program: muinvv products; chunk 0 in two sub-chunks.
        # sg counts: m0a=1, m0b=2, m1=3, m2=4.
        nc.gpsimd.wait_ge(dsem["mu0"], 16)
        nc.gpsimd.wait_ge(sa, 1)          # exp0a done
        nc.gpsimd.tensor_mul(
            muinvv.ap()[:, SUB[0]], mu.ap()[:, SUB[0]], invv.ap()[:, SUB[0]]
        ).then_inc(sg, 1)
        nc.gpsimd.wait_ge(sa, 2)          # exp0b done
        nc.gpsimd.tensor_mul(
            muinvv.ap()[:, SUB[1]], mu.ap()[:, SUB[1]], invv.ap()[:, SUB[1]]
        ).then_inc(sg, 1)
        for h in (1, 2):
            nc.gpsimd.wait_ge(dsem[f"mu{h}"], 16)
            nc.gpsimd.wait_ge(sa, 2 * h + 2)  # exp_h done
            nc.gpsimd.tensor_mul(
                muinvv.ap()[:, sls[h]], mu.ap()[:, sls[h]],
                invv.ap()[:, sls[h]]
            ).then_inc(sg, 1)

        # DVE program: per chunk Sx, A, C, B (baseline order).
        for h in range(NCH):
            nc.vector.wait_ge(dsem[f"x{h}"], 16)
            nc.vector.tensor_scalar(
                out=jds[("Sx", h)].ap()[:, :], in0=xb.ap()[:, sls[h]],
                scalar1=1.0, scalar2=0.0, op0=M, op1=mybir.AluOpType.add,
                accum_out=col("Sx", h),
            ).then_inc(sv, 1)
            nc.vector.wait_ge(sa, 2 * h + 3)  # sq_h done => x2_h, invv_h
            nc.vector.scalar_tensor_tensor(
                out=jds[("A", h)].ap()[:, :], in0=x2.ap()[:, sls[h]], scalar=1.0,
                in1=invv.ap()[:, sls[h]], op0=M, op1=M,
                accum_out=col("A", h),
            ).then_inc(sv, 1)
            nc.vector.wait_ge(sg, h + 2)      # full muinvv_h done
            nc.vector.scalar_tensor_tensor(
                out=jds[("C", h)].ap()[:, :], in0=mu.ap()[:, sls[h]], scalar=1.0,
                in1=muinvv.ap()[:, sls[h]], op0=M, op1=M,
                accum_out=col("C", h),
            ).then_inc(sv, 1)
            nc.vector.scalar_tensor_tensor(
                out=jds[("B", h)].ap()[:, :], in0=xb.ap()[:, sls[h]], scalar=1.0,
                in1=muinvv.ap()[:, sls[h]], op0=M, op1=M,
                accum_out=col("B", h),
            ).then_inc(sv, 1)

        # Output: wait for every accumulator column, ship, then confirm.
        nc.sync.wait_ge(sa, 3 * NCH + 1)
        nc.sync.wait_ge(sv, 4 * NCH)
        nc.sync.dma_start(out=accs[:, :], in_=acc.ap()[:, :]).then_inc(so, 16)
        nc.sync.wait_ge(so, 16)
        nc.all_engine_barrier()
    return nc


def _ensure_ntff_hook():
    import types

    if "antenv.axon_hooks" in sys.modules:
        return
    try:
        import antenv.axon_hooks  # noqa: F401
        return
    except ImportError:
        pass
    hook = None
    try:
        sys.path.insert(0, "/root/.axon_site")
        from trn_agent_boot.trn_boot import _ntff_profile_via_ctypes

        hook = _ntff_profile_via_ctypes("/opt/axon/libaxon_pjrt.so")
    except Exception:
        hook = None
    mod = types.ModuleType("antenv.axon_hooks")
    mod._hook = hook
    mod.get_axon_ntff_profile_hook = lambda: mod._hook
    mod.set_axon_ntff_profile_hook = lambda h: setattr(mod, "_hook", h)
    sys.modules["antenv.axon_hooks"] = mod


_ensure_ntff_hook()

_NC = None


def _get_nc():
    global _NC
    if _NC is None:
        _NC = build_nc()
        _NC.compile()
    return _NC


def make_in_maps(x, mu, logvar):
    x = np.ascontiguousarray(np.asarray(x, dtype=np.float32))
    mu = np.asarray(mu, dtype=np.float32)
    lv = np.asarray(logvar, dtype=np.float32)
    in_maps = []
    for c in range(N_CORES):
        r0 = c * ROWS
        mu_t = np.concatenate(
            [mu[r0 + b * HW:r0 + (b + 1) * HW].T for b in range(NB)], axis=0
        )
        lv_t = np.concatenate(
            [lv[r0 + b * HW:r0 + (b + 1) * HW].T for b in range(NB)], axis=0
        )
        x_t = x[c * NB:(c + 1) * NB].reshape(128, COLS)
        m = {}
        for h in range(len(BOUNDS) - 1):
            sl = slice(BOUNDS[h], BOUNDS[h + 1])
            m[f"xn{h}"] = np.ascontiguousarray(x_t[:, sl])
            m[f"mut{h}"] = np.ascontiguousarray(mu_t[:, sl])
            m[f"lvt{h}"] = np.ascontiguousarray(lv_t[:, sl])
        in_maps.append(m)
    return in_maps


def combine(results) -> np.ndarray:
    nq = len(QUANT)
    tot = np.zeros((nq, 128), dtype=np.float64)
    for r in results:
        a = np.asarray(r["accs"], dtype=np.float64)
        for q in range(nq):
            tot[q] += a[:, q * NCH:(q + 1) * NCH].sum(axis=1)
        tot[QUANT.index("Sinvv")] += a[:, 21]  # exp0b sub-chunk column
    scal = {q: tot[i].sum() for i, q in enumerate(QUANT[:3])}
    vec = {q: tot[i].reshape(NB, D).sum(axis=0)
           for i, q in enumerate(QUANT) if i >= 3}
    A, Bs, C = scal["A"], scal["B"], scal["C"]
    mean_pos = -0.5 / N * (A - 2.0 * Bs + C)
    mean_D = (vec["Sinvv"] @ vec["Sx2"] - 2.0 * vec["Smuinvv"] @ vec["Sx"]
              + N * C) / float(N) ** 2
    loss = mean_pos + 0.5 * mean_D
    return np.array(loss, dtype=np.float32)


def kernel(x, mu, logvar, **_kwargs):
    nc = _get_nc()
    in_maps = make_in_maps(x, mu, logvar)
    res = run_bass_kernel_spmd(nc, in_maps, list(range(N_CORES)))
    return combine(res.results)
